# revision 29
# baseline (speedup 1.0000x reference)
"""AdaptiveFractalFeedForward Trainium2 kernel (8 NeuronCores).

Strategy:
  - Token sharding: core c owns the tokens of one expert level plus a
    filler slice of a "split" level, exactly 512 tokens. The adapter
    therefore operates on the SAME per-core token set as the main MLP:
    slot0 (own level) covers columns [0, cap0), slot1 (split level)
    covers [base1, 512). Windows overlap; the host discards the
    columns that don't belong to each slot.
  - Main MLP bf16; adapter fp8(e4m3) with DoubleRow matmuls (2x PE)
    for slot0. Adapter output scales by mix ~5e-4 so fp8 error is
    negligible. Adapter weights pre-scaled by 8 (dodges fp8
    subnormals), un-scaled via mix/64.
  - LayerNorm without transposes: host sends x^T (feature-major);
    token mean/var come from PE ones-matmuls (sums of x and x^2 land
    pre-broadcast across partitions; eps via a K=1 accumulate).
    rstd = bf16 bit-trick rsqrt + 1 Newton step on the DVE; normalize
    is 2 batched DVE ops. Device computes -x_norm; host negates W1/A1.
  - PE warmup matmuls + warm-keeper trickle so the HAM clock gate
    stays at 2.4GHz through the stats chain.
  - bf16 partial outputs; host combines in fp32.
"""

import math
from contextlib import ExitStack

import ml_dtypes
import numpy as np

import concourse.bass as bass
import concourse.mybir as mybir
import concourse.tile as tile
from concourse import bacc
from concourse.bass_utils import run_bass_kernel_spmd
from concourse.tile_rust import add_dep_helper

B, S, D = 2, 2048, 768
HID, HID2 = 3072, 1536
NLEV = 9
NCORES = 8
TPC = (B * S) // NCORES  # 512 tokens per core
P = 128
KD = D // P        # 6
KH = HID // P      # 24
KH2 = HID2 // P    # 12
EPS = 1e-5
ASCALE = 8.0       # adapter weight pre-scale
NWARM = 8          # PE warmup matmuls
MAGIC16 = 0x5F37   # bf16 rsqrt magic

F32 = mybir.dt.float32
BF16 = mybir.dt.bfloat16
F8 = mybir.dt.float8e4
I16 = mybir.dt.int16
AF = mybir.ActivationFunctionType
AO = mybir.AluOpType
DR = mybir.MatmulPerfMode.DoubleRow

_PROGRAM_CACHE: dict = {}
LAST_EXEC_NS = None
LAST_RESULTS = None


def _build_program(cap0: int, base1: int, cap1: int):
    assert cap0 <= TPC and base1 + cap1 == TPC
    wout = TPC + cap0 + cap1

    nc = bacc.Bacc("TRN2", target_bir_lowering=False, debug=False,
                   num_devices=NCORES)

    xmT = nc.dram_tensor("xmT", [P, KD, TPC], BF16, kind="ExternalInput").ap()
    w1 = nc.dram_tensor("W1", [D, HID], BF16, kind="ExternalInput").ap()
    # W2 host-pretiled: [dt, p, kk, di] = W2[kk*128+p, dt*128+di]
    w2t = nc.dram_tensor("W2t", [KD, P, KH, P], BF16,
                         kind="ExternalInput").ap()
    # A1 host layout: [s, p, kk, h] = -8*A1_eff[s][kk*128+p, h]  (fp8)
    a1g = nc.dram_tensor("A1g", [2, P, KD, HID2], F8,
                         kind="ExternalInput").ap()
    # A2 host layout: [s, p, dt, kk, m] = 8*A2[s][kk*128+p, dt*128+m] (fp8)
    a2gt = nc.dram_tensor("A2gt", [2, P, KD, KH2, P], F8,
                          kind="ExternalInput").ap()
    b1v = nc.dram_tensor("b1", [P, KH], F32, kind="ExternalInput").ap()
    b2v = nc.dram_tensor("b2", [P, KD], F32, kind="ExternalInput").ap()
    a1bg = nc.dram_tensor("a1bg", [P, 2, KH2], F32, kind="ExternalInput").ap()
    a2bg = nc.dram_tensor("a2bg", [P, 2, KD], F32, kind="ExternalInput").ap()
    ommb = nc.dram_tensor("ommb", [P, TPC], F32, kind="ExternalInput").ap()
    mixab = nc.dram_tensor("mixab", [P, TPC], F32, kind="ExternalInput").ap()
    out = nc.dram_tensor("out", [D, wout], BF16, kind="ExternalOutput").ap()

    segs0 = [(0, cap0)]
    segs1 = [(base1, cap1)]

    with tile.TileContext(nc) as tc, ExitStack() as ctx, \
            nc.allow_low_precision(reason="bf16 LN stats are within budget"):
        singles = ctx.enter_context(tc.tile_pool(name="singles", bufs=1))
        wpool = ctx.enter_context(tc.tile_pool(name="wpool", bufs=3))
        w2pool = ctx.enter_context(tc.tile_pool(name="w2pool", bufs=6))
        opool = ctx.enter_context(tc.tile_pool(name="opool", bufs=4))
        sqpool = ctx.enter_context(tc.tile_pool(name="sqpool", bufs=1))
        vpool = ctx.enter_context(tc.tile_pool(name="vpool", bufs=2))
        pacc = ctx.enter_context(tc.tile_pool(name="pacc", bufs=3,
                                              space="PSUM"))
        pout = ctx.enter_context(tc.tile_pool(name="pout", bufs=3,
                                              space="PSUM"))
        psum_s = ctx.enter_context(tc.tile_pool(name="psum_s", bufs=2,
                                                space="PSUM"))

        def chain(op, prev, why="queue order"):
            if prev is not None:
                add_dep_helper(op.ins, prev.ins, reason=why)
            return op

        def bmid(ap, n):
            """Broadcast a [P, W] AP across a middle free dim of size n."""
            return bass.AP(tensor=ap.tensor, offset=ap.offset,
                           ap=[ap.ap[0], [0, n], ap.ap[1]])

        # ---- earliest DMA: xmT alone on the SP ring (2 unchained chunks
        # so squares can start on chunk 0) ----
        CHUNKS = [6, 6, 6, 6]
        w1_r = w1.rearrange("(t p) h -> p t h", p=P)

        xmT_sb = singles.tile([P, KD, TPC], BF16)
        nc.sync.dma_start(out=xmT_sb[:, 0:3, :], in_=xmT[:, 0:3, :])
        d_xmt = nc.sync.dma_start(out=xmT_sb[:, 3:6, :], in_=xmT[:, 3:6, :])
        d_sp = d_xmt

        # SWDGE: W1c0 held until xmT is through, then smalls
        w1c0 = wpool.tile([P, KD, 6 * P], BF16, tag="wa")
        d_gp = chain(nc.gpsimd.dma_start(out=w1c0, in_=w1_r[:, :, 0:6 * P]),
                     d_xmt, "W1c0 after xmT (startup bw)")
        b1_sb = singles.tile([P, KH], F32)
        d_gp = chain(nc.gpsimd.dma_start(out=b1_sb, in_=b1v), d_gp)
        b2_sb = singles.tile([P, KD], F32)
        d_gp = chain(nc.gpsimd.dma_start(out=b2_sb, in_=b2v), d_gp)
        a1b_sb = singles.tile([P, 2, KH2], F32)
        d_gp = chain(nc.gpsimd.dma_start(out=a1b_sb, in_=a1bg), d_gp)
        a2b_sb = singles.tile([P, 2, KD], F32)
        d_gp = chain(nc.gpsimd.dma_start(out=a2b_sb, in_=a2bg), d_gp)
        omm_sb = singles.tile([P, TPC], F32)
        d_gp = chain(nc.gpsimd.dma_start(out=omm_sb, in_=ommb), d_gp)
        mixa_sb = singles.tile([P, TPC], F32)
        d_gp = chain(nc.gpsimd.dma_start(out=mixa_sb, in_=mixab), d_gp)

        # ---- PE warmup ----
        # ones_t doubles as the 2^-10-scaled summing vector for LN stats
        ones_t = singles.tile([P, P], BF16)
        nc.vector.memset(ones_t, 2.0 ** -10)
        # eps row for the K=1 accumulate: eps*(3/4)*1024
        edm = singles.tile([1, TPC], BF16)
        nc.vector.memset(edm, EPS * 0.75 * 1024.0)
        warm_t = singles.tile([P, 256], BF16)
        nc.vector.memset(warm_t, 0.125)
        for i in range(NWARM):
            wp = pacc.tile([P, 256], F32, tag="acc")
            nc.tensor.matmul(wp, warm_t[:, 0:P], warm_t, start=True,
                             stop=True)

        # persistent activations
        xm_t = singles.tile([P, KD, TPC], BF16)   # -x_norm^T (bf16)
        xa8 = singles.tile([P, KD, TPC], F8)      # -x_norm^T (fp8)
        h_sb = singles.tile([P, KH, TPC], BF16)   # gelu(h)
        hl_sb = singles.tile([P, KH2, TPC], F8)   # relu(hl)*8 (fp8)

        C43 = 1024.0 / D  # sums are pre-scaled by 2^-10 via ones_t

        def trickle(after):
            wp2 = pacc.tile([P, 256], F32, tag="acc")
            wmm = nc.tensor.matmul(wp2, warm_t[:, 0:P], warm_t, start=True,
                                   stop=True)
            add_dep_helper(wmm.ins, after.ins, reason="HAM warm-keeper")

        # ---- LN stats + normalize: xm_t = (m - x) * rstd  (negated) ----
        rs_bc = singles.tile([P, TPC], BF16)
        tmp_n = sqpool.tile([P, KD, TPC], BF16, tag="nt")
        sq = sqpool.tile([P, KD, TPC], BF16, tag="sq")
        v0 = nc.vector.tensor_mul(out=sq[:, 0:3, :], in0=xmT_sb[:, 0:3, :],
                                  in1=xmT_sb[:, 0:3, :])
        v0 = chain(nc.vector.tensor_mul(out=sq[:, 3:6, :],
                                        in0=xmT_sb[:, 3:6, :],
                                        in1=xmT_sb[:, 3:6, :]), v0)
        ps1 = psum_s.tile([P, TPC], F32, tag="s")
        ps2 = psum_s.tile([P, TPC], F32, tag="s")
        for kk in range(KD):
            nc.tensor.matmul(ps1, ones_t, xmT_sb[:, kk, :],
                             start=(kk == 0), stop=(kk == KD - 1))
            nc.tensor.matmul(ps2, ones_t, sq[:, kk, :],
                             start=(kk == 0), stop=False)
        xm_sum_last = nc.tensor.matmul(ps2, ones_t[0:1, :], edm[0:1, :],
                                       start=False, stop=True)
        # m, E[x^2] to bf16 SBUF on ACT (keeps the DVE chain in 2x mode)
        m_sb = vpool.tile([P, TPC], BF16, tag="vb")
        nc.scalar.activation(out=m_sb, in_=ps1, func=AF.Copy, scale=C43)
        v_t = vpool.tile([P, TPC], BF16, tag="v")
        nc.scalar.activation(out=v_t, in_=ps2, func=AF.Copy, scale=C43)
        msq = vpool.tile([P, TPC], BF16, tag="vb")
        last = chain(nc.vector.tensor_mul(out=msq, in0=m_sb, in1=m_sb), v0)
        last = chain(nc.vector.tensor_sub(out=v_t, in0=v_t, in1=msq), last)
        trickle(last)
        # rsqrt bit-trick on bf16 bits + 1 Newton step
        y = vpool.tile([P, TPC], BF16, tag="y")
        last = chain(nc.vector.tensor_scalar(
            out=y.bitcast(I16), in0=v_t.bitcast(I16), scalar1=1,
            scalar2=None, op0=AO.logical_shift_right), last)
        last = chain(nc.vector.tensor_scalar(
            out=y.bitcast(I16), in0=y.bitcast(I16), scalar1=-1,
            scalar2=MAGIC16, op0=AO.mult, op1=AO.add), last)
        t1 = vpool.tile([P, TPC], BF16, tag="vb")
        last = chain(nc.vector.tensor_mul(out=t1, in0=v_t, in1=y), last)
        last = chain(nc.vector.tensor_mul(out=t1, in0=t1, in1=y), last)
        last = chain(nc.vector.tensor_scalar(out=t1, in0=t1, scalar1=-0.5,
                                             scalar2=1.5, op0=AO.mult,
                                             op1=AO.add), last)
        last = chain(nc.vector.tensor_mul(out=rs_bc, in0=y, in1=t1), last)
        trickle(last)
        # normalize (negated): xm_t = (m - x) * rs
        last = chain(nc.vector.tensor_sub(out=tmp_n, in0=bmid(m_sb, KD),
                                          in1=xmT_sb), last)
        trickle(last)
        norm_last = chain(nc.vector.tensor_mul(out=xm_t, in0=tmp_n,
                                               in1=bmid(rs_bc, KD)), last)

        # fp8 copy for the adapter (off the critical path, during h)
        x8a = chain(nc.vector.tensor_copy(out=xa8[:, 0:3, :],
                                          in_=xm_t[:, 0:3, :]), norm_last)
        chain(nc.vector.tensor_copy(out=xa8[:, 3:6, :],
                                    in_=xm_t[:, 3:6, :]), x8a)

        # ---- remaining W1 on SP/ACT rings ----
        w1c1 = wpool.tile([P, KD, 6 * P], BF16, tag="wa")
        d_sp = chain(nc.sync.dma_start(out=w1c1,
                                       in_=w1_r[:, :, 6 * P:12 * P]), d_sp)

        # ---- phase A1: h = gelu(x_norm @ W1 + b1) ----
        W1ENG = [None, None, "scalar", "scalar"]
        d_act = None
        ht = 0
        gelu_first = None
        for ci, nch in enumerate(CHUNKS):
            if ci == 0:
                w1c = w1c0
            elif ci == 1:
                w1c = w1c1
            else:
                w1c = wpool.tile([P, KD, 6 * P], BF16, tag="wa")
                eng = getattr(nc, W1ENG[ci])
                c0 = sum(CHUNKS[:ci])
                dma = eng.dma_start(out=w1c[:, :, 0:nch * P],
                                    in_=w1_r[:, :, c0 * P:(c0 + nch) * P])
                if ci == 2:
                    add_dep_helper(dma.ins, d_xmt.ins,
                                   reason="W1c2 after xmT (startup bw)")
                else:
                    chain(dma, d_act)
                d_act = dma
            for j in range(nch):
                h_ps = pacc.tile([P, TPC], F32, tag="acc")
                for k in range(KD):
                    nc.tensor.matmul(h_ps, w1c[:, k, j * P:(j + 1) * P],
                                     xm_t[:, k, :], start=(k == 0),
                                     stop=(k == KD - 1))
                g = nc.scalar.activation(out=h_sb[:, ht, :], in_=h_ps,
                                         func=AF.Gelu,
                                         bias=b1_sb[:, ht:ht + 1])
                if gelu_first is None:
                    gelu_first = g
                ht += 1

        # ---- W2 loads (SP ring, all resident) ----
        w2cs = []
        for dt in range(KD):
            w2c = w2pool.tile([P, KH, P], BF16, tag="w2")
            d_sp = chain(nc.sync.dma_start(out=w2c, in_=w2t[dt]), d_sp)
            w2cs.append(w2c)

        # ---- A1 loads (SWDGE, held until the h phase is underway) ----
        a1_sb = singles.tile([P, 2, KD, HID2], F8)
        for s in range(2):
            d_gp = chain(nc.gpsimd.dma_start(out=a1_sb[:, s], in_=a1g[s]),
                         d_gp if s else gelu_first,
                         "A1 after h start (startup bw)")

        # ---- phase A2: main_out = (h @ W2 + b2) * (1-mix) ----
        for dt in range(KD):
            o_ps = pout.tile([P, TPC], F32, tag="po")
            for kk in range(KH):
                nc.tensor.matmul(o_ps, w2cs[dt][:, kk, :], h_sb[:, kk, :],
                                 start=(kk == 0), stop=(kk == KH - 1))
            o_sb = opool.tile([P, TPC], BF16, tag="osb")
            nc.vector.scalar_tensor_tensor(out=o_sb, in0=o_ps,
                                           scalar=b2_sb[:, dt:dt + 1],
                                           in1=omm_sb, op0=AO.add,
                                           op1=AO.mult)
            nc.scalar.dma_start(out=out[dt * P:(dt + 1) * P, 0:TPC], in_=o_sb)

        # ---- A2 prefetch (SP ring, after W2) ----
        a2_sb = singles.tile([P, 2, KD, KH2, P], F8)
        for s in range(2):
            d_sp = chain(nc.sync.dma_start(out=a2_sb[:, s], in_=a2gt[s]),
                         d_sp)

        # ---- phase B1: hl = relu(x_norm @ A1*8 + 8*a1b) ----
        for ht2 in range(KH2):
            for (sb, sl) in segs0:
                hl_ps = pacc.tile([P, TPC], F32, tag="acc")
                for k in range(KD // 2):
                    nc.tensor.matmul(
                        hl_ps[:, 0:sl],
                        a1_sb[:, 0, 2 * k:2 * k + 2, ht2 * P:(ht2 + 1) * P],
                        xa8[:, 2 * k:2 * k + 2, sb:sb + sl],
                        start=(k == 0), stop=(k == KD // 2 - 1),
                        perf_mode=DR)
                nc.scalar.activation(out=hl_sb[:, ht2, sb:sb + sl],
                                     in_=hl_ps[:, 0:sl], func=AF.Relu,
                                     bias=a1b_sb[:, 0, ht2:ht2 + 1])
            for (sb, sl) in segs1:
                hl_ps = pacc.tile([P, TPC], F32, tag="acc")
                for k in range(KD):
                    nc.tensor.matmul(hl_ps[:, 0:sl],
                                     a1_sb[:, 1, k, ht2 * P:(ht2 + 1) * P],
                                     xa8[:, k, sb:sb + sl],
                                     start=(k == 0), stop=(k == KD - 1))
                nc.scalar.activation(out=hl_sb[:, ht2, sb:sb + sl],
                                     in_=hl_ps[:, 0:sl], func=AF.Relu,
                                     bias=a1b_sb[:, 1, ht2:ht2 + 1])

        # ---- phase B2: adapter_out = (hl @ A2*8 + 64*a2b) * (mix/64) ----
        for dt in range(KD):
            for si, segs in enumerate((segs0, segs1)):
                col0 = TPC if si == 0 else TPC + cap0 - base1
                for (sb, sl) in segs:
                    ao_ps = pout.tile([P, TPC], F32, tag="po")
                    if si == 0:
                        for k in range(KH2 // 2):
                            nc.tensor.matmul(
                                ao_ps[:, 0:sl],
                                a2_sb[:, 0, dt, 2 * k:2 * k + 2, :],
                                hl_sb[:, 2 * k:2 * k + 2, sb:sb + sl],
                                start=(k == 0), stop=(k == KH2 // 2 - 1),
                                perf_mode=DR)
                    else:
                        for kk in range(KH2):
                            nc.tensor.matmul(ao_ps[:, 0:sl],
                                             a2_sb[:, 1, dt, kk, :],
                                             hl_sb[:, kk, sb:sb + sl],
                                             start=(kk == 0),
                                             stop=(kk == KH2 - 1))
                    ao_sb = opool.tile([P, TPC], BF16, tag="osb")
                    nc.vector.scalar_tensor_tensor(
                        out=ao_sb[:, 0:sl], in0=ao_ps[:, 0:sl],
                        scalar=a2b_sb[:, si, dt:dt + 1],
                        in1=mixa_sb[:, sb:sb + sl], op0=AO.add, op1=AO.mult)
                    (nc.sync if si == 0 else nc.scalar).dma_start(
                        out=out[dt * P:(dt + 1) * P,
                                col0 + sb:col0 + sb + sl],
                        in_=ao_sb[:, 0:sl])

    nc.compile()
    return nc


def kernel(x, levels_info, gamma, beta, W1, b1, W2, b2, A1, a1b, A2, a2b,
           lmw, _trace=False, _trace_kwargs=None):
    global LAST_EXEC_NS, LAST_RESULTS
    x = np.ascontiguousarray(np.asarray(x, dtype=np.float32))
    levels_info = np.asarray(levels_info)
    gamma = np.asarray(gamma, dtype=np.float32)
    beta = np.asarray(beta, dtype=np.float32)
    W1 = np.asarray(W1, dtype=np.float32)
    b1 = np.asarray(b1, dtype=np.float32)
    W2 = np.asarray(W2, dtype=np.float32)
    b2 = np.asarray(b2, dtype=np.float32)
    A1 = np.asarray(A1, dtype=np.float32)
    a1b = np.asarray(a1b, dtype=np.float32)
    A2 = np.asarray(A2, dtype=np.float32)
    a2b = np.asarray(a2b, dtype=np.float32)
    lmw = np.asarray(lmw, dtype=np.float32)

    bf = ml_dtypes.bfloat16
    f8 = ml_dtypes.float8_e4m3

    xflat = x.reshape(B * S, D)

    # softmax over the sequence axis of lmw[depths] (shared across batch)
    depths = np.clip(levels_info[:, 0].astype(np.int64), 0, NLEV - 1)
    vals = lmw[depths]
    e = np.exp((vals - vals.max()).astype(np.float32))
    mix_pos = (e / e.sum()).astype(np.float32)  # [S]
    mix_flat = np.concatenate([mix_pos, mix_pos])  # [B*S]
    dflat = np.concatenate([depths, depths])

    # ---- pick the split level minimizing the slot-window overlap ----
    buckets = [np.nonzero(dflat == l)[0] for l in range(NLEV)]
    sizes = np.array([len(b) for b in buckets])
    best = None
    for l in range(NLEV):
        others = [i for i in range(NLEV) if i != l]
        mx = int(sizes[others].max())
        mn = int(sizes[others].min())
        if mx > TPC or mn == 0:
            continue
        cost = mx + TPC - mn  # cap0 + cap1
        if best is None or cost < best[0]:
            best = (cost, l, mx, mn)
    _, lsplit, cap0, base1 = best
    cap1 = TPC - base1
    others = [i for i in range(NLEV) if i != lsplit]

    key = (cap0, base1, cap1)
    if key not in _PROGRAM_CACHE:
        _PROGRAM_CACHE[key] = _build_program(cap0, base1, cap1)
    nc = _PROGRAM_CACHE[key]

    # ---- shared (replicated) host tensors ----
    # LayerNorm affine folded into first-layer weights; device computes
    # MINUS x_norm, so the x-side weights are negated.
    w1_eff = gamma[:, None] * W1
    b1_eff = (b1 + beta @ W1).astype(np.float32)
    w1_host = (-w1_eff).astype(bf)
    w2t_host = np.ascontiguousarray(
        W2.reshape(KH, P, KD, P).transpose(2, 1, 0, 3).astype(bf))
    A1_eff = -(gamma[None, :, None] * A1) * ASCALE
    a1b_eff = ((a1b + np.einsum("d,ldh->lh", beta, A1)) * ASCALE).astype(
        np.float32)
    A2_s = A2 * ASCALE
    a2b_s = (a2b * ASCALE * ASCALE).astype(np.float32)

    xflat_bf = xflat.astype(bf)
    b1_host = np.ascontiguousarray(b1_eff.reshape(KH, P).T)
    b2_host = np.ascontiguousarray(b2.reshape(KD, P).T)

    def xT_pack(tokens_2d):
        # [N, D] -> [P, KD, N] with [p, kk, t] = x[t, kk*128+p]
        n = tokens_2d.shape[0]
        return np.ascontiguousarray(
            tokens_2d.T.reshape(KD, P, n).transpose(1, 0, 2))

    # partition the split level into per-core fills
    fill_needed = [TPC - int(sizes[others[c]]) for c in range(NCORES)]
    assert sum(fill_needed) == int(sizes[lsplit])
    fill_tok = []
    off = 0
    for c in range(NCORES):
        fill_tok.append(buckets[lsplit][off:off + fill_needed[c]])
        off += fill_needed[c]

    in_maps = []
    scatters = []
    for c in range(NCORES):
        lvl0 = int(others[c])
        tok0 = buckets[lvl0]
        tok1 = fill_tok[c]
        toks = np.concatenate([tok0, tok1])
        len0 = len(tok0)

        a1g_c = np.ascontiguousarray(np.stack([
            A1_eff[lvl0].reshape(KD, P, HID2).transpose(1, 0, 2),
            A1_eff[lsplit].reshape(KD, P, HID2).transpose(1, 0, 2),
        ]).astype(f8))  # [2, P, KD, HID2]
        a2g_c = np.ascontiguousarray(np.stack([
            A2_s[lvl0].reshape(KH2, P, KD, P).transpose(1, 2, 0, 3),
            A2_s[lsplit].reshape(KH2, P, KD, P).transpose(1, 2, 0, 3),
        ]).astype(f8))  # [2, P, KD, KH2, P]

        in_maps.append({
            "xmT": xT_pack(xflat_bf[toks]),
            "W1": w1_host,
            "W2t": w2t_host,
            "A1g": a1g_c,
            "A2gt": a2g_c,
            "b1": b1_host,
            "b2": b2_host,
            "a1bg": np.ascontiguousarray(
                np.stack([a1b_eff[lvl0], a1b_eff[lsplit]])
                .reshape(2, KH2, P).transpose(2, 0, 1).astype(np.float32)),
            "a2bg": np.ascontiguousarray(
                np.stack([a2b_s[lvl0], a2b_s[lsplit]])
                .reshape(2, KD, P).transpose(2, 0, 1).astype(np.float32)),
            "ommb": np.ascontiguousarray(np.broadcast_to(
                (1.0 - mix_flat[toks]).astype(np.float32), (P, TPC))),
            "mixab": np.ascontiguousarray(np.broadcast_to(
                (mix_flat[toks] / (ASCALE * ASCALE)).astype(np.float32),
                (P, TPC))),
        })
        scatters.append((toks, tok0, tok1, len0))

    res = run_bass_kernel_spmd(nc, in_maps, core_ids=list(range(NCORES)),
                               trace=_trace, **(_trace_kwargs or {}))
    LAST_EXEC_NS = res.exec_time_ns
    LAST_RESULTS = res

    # ---- unshard: main part + additive adapter part ----
    result = np.zeros((B * S, D), dtype=np.float32)
    for c in range(NCORES):
        o = np.asarray(res.results[c]["out"]).astype(np.float32)
        toks, tok0, tok1, len0 = scatters[c]
        result[toks] = o[:, :TPC].T
        if len0:
            result[tok0] += o[:, TPC:TPC + len0].T
        if len(tok1):
            s1 = TPC + cap0 + (len0 - base1)
            result[tok1] += o[:, s1:s1 + len(tok1)].T
    return result.reshape(B, S, D)


# revision 30
# speedup vs baseline: 1.0055x; 1.0055x over previous
"""AdaptiveFractalFeedForward Trainium2 kernel (8 NeuronCores).

Strategy:
  - Token sharding: core c owns the tokens of one expert level plus a
    filler slice of a "split" level, exactly 512 tokens. The adapter
    therefore operates on the SAME per-core token set as the main MLP:
    slot0 (own level) covers columns [0, cap0), slot1 (split level)
    covers [base1, 512). Windows overlap; the host discards the
    columns that don't belong to each slot.
  - Main MLP bf16; adapter fp8(e4m3) with DoubleRow matmuls (2x PE)
    for slot0. Adapter output scales by mix ~5e-4 so fp8 error is
    negligible. Adapter weights pre-scaled by 8 (dodges fp8
    subnormals), un-scaled via mix/64.
  - LayerNorm without transposes: host sends x^T (feature-major);
    token mean/var come from PE ones-matmuls (sums of x and x^2 land
    pre-broadcast across partitions; eps via a K=1 accumulate).
    rstd = bf16 bit-trick rsqrt + 1 Newton step on the DVE; normalize
    is 2 batched DVE ops. Device computes -x_norm; host negates W1/A1.
  - PE warmup matmuls + warm-keeper trickle so the HAM clock gate
    stays at 2.4GHz through the stats chain.
  - bf16 partial outputs; host combines in fp32.
"""

import math
from contextlib import ExitStack

import ml_dtypes
import numpy as np

import concourse.bass as bass
import concourse.mybir as mybir
import concourse.tile as tile
from concourse import bacc
from concourse.bass_utils import run_bass_kernel_spmd
from concourse.tile_rust import add_dep_helper

B, S, D = 2, 2048, 768
HID, HID2 = 3072, 1536
NLEV = 9
NCORES = 8
TPC = (B * S) // NCORES  # 512 tokens per core
P = 128
KD = D // P        # 6
KH = HID // P      # 24
KH2 = HID2 // P    # 12
EPS = 1e-5
ASCALE = 8.0       # adapter weight pre-scale
NWARM = 11         # PE warmup matmuls
MAGIC16 = 0x5F37   # bf16 rsqrt magic

F32 = mybir.dt.float32
BF16 = mybir.dt.bfloat16
F8 = mybir.dt.float8e4
I16 = mybir.dt.int16
AF = mybir.ActivationFunctionType
AO = mybir.AluOpType
DR = mybir.MatmulPerfMode.DoubleRow

_PROGRAM_CACHE: dict = {}
LAST_EXEC_NS = None
LAST_RESULTS = None


def _build_program(cap0: int, base1: int, cap1: int):
    assert cap0 <= TPC and base1 + cap1 == TPC
    wout = TPC + cap0 + cap1

    nc = bacc.Bacc("TRN2", target_bir_lowering=False, debug=False,
                   num_devices=NCORES)

    xmT = nc.dram_tensor("xmT", [P, KD, TPC], BF16, kind="ExternalInput").ap()
    w1 = nc.dram_tensor("W1", [D, HID], BF16, kind="ExternalInput").ap()
    # W2 host-pretiled: [dt, p, kk, di] = W2[kk*128+p, dt*128+di]
    w2t = nc.dram_tensor("W2t", [KD, P, KH, P], BF16,
                         kind="ExternalInput").ap()
    # A1 host layout: [s, p, kk, h] = -8*A1_eff[s][kk*128+p, h]  (fp8)
    a1g = nc.dram_tensor("A1g", [2, P, KD, HID2], F8,
                         kind="ExternalInput").ap()
    # A2 host layout: [s, p, dt, kk, m] = 8*A2[s][kk*128+p, dt*128+m] (fp8)
    a2gt = nc.dram_tensor("A2gt", [2, P, KD, KH2, P], F8,
                          kind="ExternalInput").ap()
    b1v = nc.dram_tensor("b1", [P, KH], F32, kind="ExternalInput").ap()
    b2v = nc.dram_tensor("b2", [P, KD], F32, kind="ExternalInput").ap()
    a1bg = nc.dram_tensor("a1bg", [P, 2, KH2], F32, kind="ExternalInput").ap()
    a2bg = nc.dram_tensor("a2bg", [P, 2, KD], F32, kind="ExternalInput").ap()
    ommb = nc.dram_tensor("ommb", [P, TPC], F32, kind="ExternalInput").ap()
    mixab = nc.dram_tensor("mixab", [P, TPC], F32, kind="ExternalInput").ap()
    out = nc.dram_tensor("out", [D, wout], BF16, kind="ExternalOutput").ap()

    segs0 = [(0, cap0)]
    segs1 = [(base1, cap1)]

    with tile.TileContext(nc) as tc, ExitStack() as ctx, \
            nc.allow_low_precision(reason="bf16 LN stats are within budget"):
        singles = ctx.enter_context(tc.tile_pool(name="singles", bufs=1))
        wpool = ctx.enter_context(tc.tile_pool(name="wpool", bufs=3))
        w2pool = ctx.enter_context(tc.tile_pool(name="w2pool", bufs=6))
        opool = ctx.enter_context(tc.tile_pool(name="opool", bufs=4))
        sqpool = ctx.enter_context(tc.tile_pool(name="sqpool", bufs=1))
        vpool = ctx.enter_context(tc.tile_pool(name="vpool", bufs=2))
        pacc = ctx.enter_context(tc.tile_pool(name="pacc", bufs=3,
                                              space="PSUM"))
        pout = ctx.enter_context(tc.tile_pool(name="pout", bufs=3,
                                              space="PSUM"))
        psum_s = ctx.enter_context(tc.tile_pool(name="psum_s", bufs=2,
                                                space="PSUM"))

        def chain(op, prev, why="queue order"):
            if prev is not None:
                add_dep_helper(op.ins, prev.ins, reason=why)
            return op

        def bmid(ap, n):
            """Broadcast a [P, W] AP across a middle free dim of size n."""
            return bass.AP(tensor=ap.tensor, offset=ap.offset,
                           ap=[ap.ap[0], [0, n], ap.ap[1]])

        # ---- earliest DMA: xmT alone on the SP ring (2 unchained chunks
        # so squares can start on chunk 0) ----
        CHUNKS = [6, 6, 6, 6]
        w1_r = w1.rearrange("(t p) h -> p t h", p=P)

        xmT_sb = singles.tile([P, KD, TPC], BF16)
        nc.sync.dma_start(out=xmT_sb[:, 0:3, :], in_=xmT[:, 0:3, :])
        d_xmt = nc.sync.dma_start(out=xmT_sb[:, 3:6, :], in_=xmT[:, 3:6, :])
        d_sp = d_xmt

        # SWDGE: W1c0 held until xmT is through, then smalls
        w1c0 = wpool.tile([P, KD, 6 * P], BF16, tag="wa")
        d_gp = chain(nc.gpsimd.dma_start(out=w1c0, in_=w1_r[:, :, 0:6 * P]),
                     d_xmt, "W1c0 after xmT (startup bw)")
        b1_sb = singles.tile([P, KH], F32)
        d_gp = chain(nc.gpsimd.dma_start(out=b1_sb, in_=b1v), d_gp)
        b2_sb = singles.tile([P, KD], F32)
        d_gp = chain(nc.gpsimd.dma_start(out=b2_sb, in_=b2v), d_gp)
        a1b_sb = singles.tile([P, 2, KH2], F32)
        d_gp = chain(nc.gpsimd.dma_start(out=a1b_sb, in_=a1bg), d_gp)
        a2b_sb = singles.tile([P, 2, KD], F32)
        d_gp = chain(nc.gpsimd.dma_start(out=a2b_sb, in_=a2bg), d_gp)
        omm_sb = singles.tile([P, TPC], F32)
        d_gp = chain(nc.gpsimd.dma_start(out=omm_sb, in_=ommb), d_gp)
        mixa_sb = singles.tile([P, TPC], F32)
        d_gp = chain(nc.gpsimd.dma_start(out=mixa_sb, in_=mixab), d_gp)

        # ---- PE warmup ----
        # ones_t doubles as the 2^-10-scaled summing vector for LN stats
        ones_t = singles.tile([P, P], BF16)
        nc.vector.memset(ones_t, 2.0 ** -10)
        # eps row for the K=1 accumulate: eps*(3/4)*1024
        edm = singles.tile([1, TPC], BF16)
        nc.vector.memset(edm, EPS * 0.75 * 1024.0)
        warm_t = singles.tile([P, 256], BF16)
        nc.vector.memset(warm_t, 0.125)
        for i in range(NWARM):
            wp = pacc.tile([P, 256], F32, tag="acc")
            nc.tensor.matmul(wp, warm_t[:, 0:P], warm_t, start=True,
                             stop=True)

        # persistent activations
        xm_t = singles.tile([P, KD, TPC], BF16)   # -x_norm^T (bf16)
        xa8 = singles.tile([P, KD, TPC], F8)      # -x_norm^T (fp8)
        h_sb = singles.tile([P, KH, TPC], BF16)   # gelu(h)
        hl_sb = singles.tile([P, KH2, TPC], F8)   # relu(hl)*8 (fp8)

        C43 = 1024.0 / D  # sums are pre-scaled by 2^-10 via ones_t

        def trickle(after):
            wp2 = pacc.tile([P, 256], F32, tag="acc")
            wmm = nc.tensor.matmul(wp2, warm_t[:, 0:P], warm_t, start=True,
                                   stop=True)
            add_dep_helper(wmm.ins, after.ins, reason="HAM warm-keeper")

        # ---- LN stats + normalize: xm_t = (m - x) * rstd  (negated) ----
        rs_bc = singles.tile([P, TPC], BF16)
        tmp_n = sqpool.tile([P, KD, TPC], BF16, tag="nt")
        sq = sqpool.tile([P, KD, TPC], BF16, tag="sq")
        v0 = nc.vector.tensor_mul(out=sq[:, 0:3, :], in0=xmT_sb[:, 0:3, :],
                                  in1=xmT_sb[:, 0:3, :])
        for kq in range(3, 6):
            v0 = chain(nc.vector.tensor_mul(out=sq[:, kq:kq + 1, :],
                                            in0=xmT_sb[:, kq:kq + 1, :],
                                            in1=xmT_sb[:, kq:kq + 1, :]), v0)
        ps1 = psum_s.tile([P, TPC], F32, tag="s")
        ps2 = psum_s.tile([P, TPC], F32, tag="s")
        for kk in range(KD):
            nc.tensor.matmul(ps1, ones_t, xmT_sb[:, kk, :],
                             start=(kk == 0), stop=(kk == KD - 1))
            nc.tensor.matmul(ps2, ones_t, sq[:, kk, :],
                             start=(kk == 0), stop=False)
        xm_sum_last = nc.tensor.matmul(ps2, ones_t[0:1, :], edm[0:1, :],
                                       start=False, stop=True)
        # m, E[x^2] to bf16 SBUF on ACT (keeps the DVE chain in 2x mode)
        m_sb = vpool.tile([P, TPC], BF16, tag="vb")
        nc.scalar.activation(out=m_sb, in_=ps1, func=AF.Copy, scale=C43)
        v_t = vpool.tile([P, TPC], BF16, tag="v")
        nc.scalar.activation(out=v_t, in_=ps2, func=AF.Copy, scale=C43)
        msq = vpool.tile([P, TPC], BF16, tag="vb")
        last = chain(nc.vector.tensor_mul(out=msq, in0=m_sb, in1=m_sb), v0)
        last = chain(nc.vector.tensor_sub(out=v_t, in0=v_t, in1=msq), last)
        trickle(last)
        # rsqrt bit-trick on bf16 bits + 1 Newton step
        y = vpool.tile([P, TPC], BF16, tag="y")
        last = chain(nc.vector.tensor_scalar(
            out=y.bitcast(I16), in0=v_t.bitcast(I16), scalar1=1,
            scalar2=None, op0=AO.logical_shift_right), last)
        last = chain(nc.vector.tensor_scalar(
            out=y.bitcast(I16), in0=y.bitcast(I16), scalar1=-1,
            scalar2=MAGIC16, op0=AO.mult, op1=AO.add), last)
        t1 = vpool.tile([P, TPC], BF16, tag="vb")
        last = chain(nc.vector.tensor_mul(out=t1, in0=v_t, in1=y), last)
        last = chain(nc.vector.tensor_mul(out=t1, in0=t1, in1=y), last)
        last = chain(nc.vector.tensor_scalar(out=t1, in0=t1, scalar1=-0.5,
                                             scalar2=1.5, op0=AO.mult,
                                             op1=AO.add), last)
        last = chain(nc.vector.tensor_mul(out=rs_bc, in0=y, in1=t1), last)
        trickle(last)
        # normalize (negated): xm_t = (m - x) * rs
        last = chain(nc.vector.tensor_sub(out=tmp_n, in0=bmid(m_sb, KD),
                                          in1=xmT_sb), last)
        trickle(last)
        norm_last = chain(nc.vector.tensor_mul(out=xm_t, in0=tmp_n,
                                               in1=bmid(rs_bc, KD)), last)

        # fp8 copy for the adapter (off the critical path, during h)
        x8a = chain(nc.vector.tensor_copy(out=xa8[:, 0:3, :],
                                          in_=xm_t[:, 0:3, :]), norm_last)
        chain(nc.vector.tensor_copy(out=xa8[:, 3:6, :],
                                    in_=xm_t[:, 3:6, :]), x8a)

        # ---- remaining W1 on SP/ACT rings ----
        w1c1 = wpool.tile([P, KD, 6 * P], BF16, tag="wa")
        d_sp = chain(nc.sync.dma_start(out=w1c1,
                                       in_=w1_r[:, :, 6 * P:12 * P]), d_sp)

        # ---- phase A1: h = gelu(x_norm @ W1 + b1) ----
        W1ENG = [None, None, "scalar", "scalar"]
        d_act = None
        ht = 0
        gelu_first = None
        for ci, nch in enumerate(CHUNKS):
            if ci == 0:
                w1c = w1c0
            elif ci == 1:
                w1c = w1c1
            else:
                w1c = wpool.tile([P, KD, 6 * P], BF16, tag="wa")
                eng = getattr(nc, W1ENG[ci])
                c0 = sum(CHUNKS[:ci])
                dma = eng.dma_start(out=w1c[:, :, 0:nch * P],
                                    in_=w1_r[:, :, c0 * P:(c0 + nch) * P])
                if ci == 2:
                    add_dep_helper(dma.ins, d_xmt.ins,
                                   reason="W1c2 after xmT (startup bw)")
                else:
                    chain(dma, d_act)
                d_act = dma
            for j in range(nch):
                h_ps = pacc.tile([P, TPC], F32, tag="acc")
                for k in range(KD):
                    nc.tensor.matmul(h_ps, w1c[:, k, j * P:(j + 1) * P],
                                     xm_t[:, k, :], start=(k == 0),
                                     stop=(k == KD - 1))
                g = nc.scalar.activation(out=h_sb[:, ht, :], in_=h_ps,
                                         func=AF.Gelu,
                                         bias=b1_sb[:, ht:ht + 1])
                if gelu_first is None:
                    gelu_first = g
                ht += 1

        # ---- W2 loads (SP ring, all resident) ----
        w2cs = []
        for dt in range(KD):
            w2c = w2pool.tile([P, KH, P], BF16, tag="w2")
            d_sp = chain(nc.sync.dma_start(out=w2c, in_=w2t[dt]), d_sp)
            w2cs.append(w2c)

        # ---- A1 loads (SWDGE, held until the h phase is underway) ----
        a1_sb = singles.tile([P, 2, KD, HID2], F8)
        for s in range(2):
            d_gp = chain(nc.gpsimd.dma_start(out=a1_sb[:, s], in_=a1g[s]),
                         d_gp if s else gelu_first,
                         "A1 after h start (startup bw)")

        # ---- phase A2: main_out = (h @ W2 + b2) * (1-mix) ----
        for dt in range(KD):
            o_ps = pout.tile([P, TPC], F32, tag="po")
            for kk in range(KH):
                nc.tensor.matmul(o_ps, w2cs[dt][:, kk, :], h_sb[:, kk, :],
                                 start=(kk == 0), stop=(kk == KH - 1))
            o_sb = opool.tile([P, TPC], BF16, tag="osb")
            nc.vector.scalar_tensor_tensor(out=o_sb, in0=o_ps,
                                           scalar=b2_sb[:, dt:dt + 1],
                                           in1=omm_sb, op0=AO.add,
                                           op1=AO.mult)
            nc.scalar.dma_start(out=out[dt * P:(dt + 1) * P, 0:TPC], in_=o_sb)

        # ---- A2 prefetch (SP ring, after W2) ----
        a2_sb = singles.tile([P, 2, KD, KH2, P], F8)
        for s in range(2):
            d_sp = chain(nc.sync.dma_start(out=a2_sb[:, s], in_=a2gt[s]),
                         d_sp)

        # ---- phase B1: hl = relu(x_norm @ A1*8 + 8*a1b) ----
        for ht2 in range(KH2):
            for (sb, sl) in segs0:
                hl_ps = pacc.tile([P, TPC], F32, tag="acc")
                for k in range(KD // 2):
                    nc.tensor.matmul(
                        hl_ps[:, 0:sl],
                        a1_sb[:, 0, 2 * k:2 * k + 2, ht2 * P:(ht2 + 1) * P],
                        xa8[:, 2 * k:2 * k + 2, sb:sb + sl],
                        start=(k == 0), stop=(k == KD // 2 - 1),
                        perf_mode=DR)
                nc.scalar.activation(out=hl_sb[:, ht2, sb:sb + sl],
                                     in_=hl_ps[:, 0:sl], func=AF.Relu,
                                     bias=a1b_sb[:, 0, ht2:ht2 + 1])
            for (sb, sl) in segs1:
                hl_ps = pacc.tile([P, TPC], F32, tag="acc")
                for k in range(KD):
                    nc.tensor.matmul(hl_ps[:, 0:sl],
                                     a1_sb[:, 1, k, ht2 * P:(ht2 + 1) * P],
                                     xa8[:, k, sb:sb + sl],
                                     start=(k == 0), stop=(k == KD - 1))
                nc.scalar.activation(out=hl_sb[:, ht2, sb:sb + sl],
                                     in_=hl_ps[:, 0:sl], func=AF.Relu,
                                     bias=a1b_sb[:, 1, ht2:ht2 + 1])

        # ---- phase B2: adapter_out = (hl @ A2*8 + 64*a2b) * (mix/64) ----
        for dt in range(KD):
            for si, segs in enumerate((segs0, segs1)):
                col0 = TPC if si == 0 else TPC + cap0 - base1
                for (sb, sl) in segs:
                    ao_ps = pout.tile([P, TPC], F32, tag="po")
                    if si == 0:
                        for k in range(KH2 // 2):
                            nc.tensor.matmul(
                                ao_ps[:, 0:sl],
                                a2_sb[:, 0, dt, 2 * k:2 * k + 2, :],
                                hl_sb[:, 2 * k:2 * k + 2, sb:sb + sl],
                                start=(k == 0), stop=(k == KH2 // 2 - 1),
                                perf_mode=DR)
                    else:
                        for kk in range(KH2):
                            nc.tensor.matmul(ao_ps[:, 0:sl],
                                             a2_sb[:, 1, dt, kk, :],
                                             hl_sb[:, kk, sb:sb + sl],
                                             start=(kk == 0),
                                             stop=(kk == KH2 - 1))
                    ao_sb = opool.tile([P, TPC], BF16, tag="osb")
                    nc.vector.scalar_tensor_tensor(
                        out=ao_sb[:, 0:sl], in0=ao_ps[:, 0:sl],
                        scalar=a2b_sb[:, si, dt:dt + 1],
                        in1=mixa_sb[:, sb:sb + sl], op0=AO.add, op1=AO.mult)
                    (nc.sync if si == 0 else nc.scalar).dma_start(
                        out=out[dt * P:(dt + 1) * P,
                                col0 + sb:col0 + sb + sl],
                        in_=ao_sb[:, 0:sl])

    nc.compile()
    return nc


def kernel(x, levels_info, gamma, beta, W1, b1, W2, b2, A1, a1b, A2, a2b,
           lmw, _trace=False, _trace_kwargs=None):
    global LAST_EXEC_NS, LAST_RESULTS
    x = np.ascontiguousarray(np.asarray(x, dtype=np.float32))
    levels_info = np.asarray(levels_info)
    gamma = np.asarray(gamma, dtype=np.float32)
    beta = np.asarray(beta, dtype=np.float32)
    W1 = np.asarray(W1, dtype=np.float32)
    b1 = np.asarray(b1, dtype=np.float32)
    W2 = np.asarray(W2, dtype=np.float32)
    b2 = np.asarray(b2, dtype=np.float32)
    A1 = np.asarray(A1, dtype=np.float32)
    a1b = np.asarray(a1b, dtype=np.float32)
    A2 = np.asarray(A2, dtype=np.float32)
    a2b = np.asarray(a2b, dtype=np.float32)
    lmw = np.asarray(lmw, dtype=np.float32)

    bf = ml_dtypes.bfloat16
    f8 = ml_dtypes.float8_e4m3

    xflat = x.reshape(B * S, D)

    # softmax over the sequence axis of lmw[depths] (shared across batch)
    depths = np.clip(levels_info[:, 0].astype(np.int64), 0, NLEV - 1)
    vals = lmw[depths]
    e = np.exp((vals - vals.max()).astype(np.float32))
    mix_pos = (e / e.sum()).astype(np.float32)  # [S]
    mix_flat = np.concatenate([mix_pos, mix_pos])  # [B*S]
    dflat = np.concatenate([depths, depths])

    # ---- pick the split level minimizing the slot-window overlap ----
    buckets = [np.nonzero(dflat == l)[0] for l in range(NLEV)]
    sizes = np.array([len(b) for b in buckets])
    best = None
    for l in range(NLEV):
        others = [i for i in range(NLEV) if i != l]
        mx = int(sizes[others].max())
        mn = int(sizes[others].min())
        if mx > TPC or mn == 0:
            continue
        cost = mx + TPC - mn  # cap0 + cap1
        if best is None or cost < best[0]:
            best = (cost, l, mx, mn)
    _, lsplit, cap0, base1 = best
    cap1 = TPC - base1
    others = [i for i in range(NLEV) if i != lsplit]

    key = (cap0, base1, cap1)
    if key not in _PROGRAM_CACHE:
        _PROGRAM_CACHE[key] = _build_program(cap0, base1, cap1)
    nc = _PROGRAM_CACHE[key]

    # ---- shared (replicated) host tensors ----
    # LayerNorm affine folded into first-layer weights; device computes
    # MINUS x_norm, so the x-side weights are negated.
    w1_eff = gamma[:, None] * W1
    b1_eff = (b1 + beta @ W1).astype(np.float32)
    w1_host = (-w1_eff).astype(bf)
    w2t_host = np.ascontiguousarray(
        W2.reshape(KH, P, KD, P).transpose(2, 1, 0, 3).astype(bf))
    A1_eff = -(gamma[None, :, None] * A1) * ASCALE
    a1b_eff = ((a1b + np.einsum("d,ldh->lh", beta, A1)) * ASCALE).astype(
        np.float32)
    A2_s = A2 * ASCALE
    a2b_s = (a2b * ASCALE * ASCALE).astype(np.float32)

    xflat_bf = xflat.astype(bf)
    b1_host = np.ascontiguousarray(b1_eff.reshape(KH, P).T)
    b2_host = np.ascontiguousarray(b2.reshape(KD, P).T)

    def xT_pack(tokens_2d):
        # [N, D] -> [P, KD, N] with [p, kk, t] = x[t, kk*128+p]
        n = tokens_2d.shape[0]
        return np.ascontiguousarray(
            tokens_2d.T.reshape(KD, P, n).transpose(1, 0, 2))

    # partition the split level into per-core fills
    fill_needed = [TPC - int(sizes[others[c]]) for c in range(NCORES)]
    assert sum(fill_needed) == int(sizes[lsplit])
    fill_tok = []
    off = 0
    for c in range(NCORES):
        fill_tok.append(buckets[lsplit][off:off + fill_needed[c]])
        off += fill_needed[c]

    in_maps = []
    scatters = []
    for c in range(NCORES):
        lvl0 = int(others[c])
        tok0 = buckets[lvl0]
        tok1 = fill_tok[c]
        toks = np.concatenate([tok0, tok1])
        len0 = len(tok0)

        a1g_c = np.ascontiguousarray(np.stack([
            A1_eff[lvl0].reshape(KD, P, HID2).transpose(1, 0, 2),
            A1_eff[lsplit].reshape(KD, P, HID2).transpose(1, 0, 2),
        ]).astype(f8))  # [2, P, KD, HID2]
        a2g_c = np.ascontiguousarray(np.stack([
            A2_s[lvl0].reshape(KH2, P, KD, P).transpose(1, 2, 0, 3),
            A2_s[lsplit].reshape(KH2, P, KD, P).transpose(1, 2, 0, 3),
        ]).astype(f8))  # [2, P, KD, KH2, P]

        in_maps.append({
            "xmT": xT_pack(xflat_bf[toks]),
            "W1": w1_host,
            "W2t": w2t_host,
            "A1g": a1g_c,
            "A2gt": a2g_c,
            "b1": b1_host,
            "b2": b2_host,
            "a1bg": np.ascontiguousarray(
                np.stack([a1b_eff[lvl0], a1b_eff[lsplit]])
                .reshape(2, KH2, P).transpose(2, 0, 1).astype(np.float32)),
            "a2bg": np.ascontiguousarray(
                np.stack([a2b_s[lvl0], a2b_s[lsplit]])
                .reshape(2, KD, P).transpose(2, 0, 1).astype(np.float32)),
            "ommb": np.ascontiguousarray(np.broadcast_to(
                (1.0 - mix_flat[toks]).astype(np.float32), (P, TPC))),
            "mixab": np.ascontiguousarray(np.broadcast_to(
                (mix_flat[toks] / (ASCALE * ASCALE)).astype(np.float32),
                (P, TPC))),
        })
        scatters.append((toks, tok0, tok1, len0))

    res = run_bass_kernel_spmd(nc, in_maps, core_ids=list(range(NCORES)),
                               trace=_trace, **(_trace_kwargs or {}))
    LAST_EXEC_NS = res.exec_time_ns
    LAST_RESULTS = res

    # ---- unshard: main part + additive adapter part ----
    result = np.zeros((B * S, D), dtype=np.float32)
    for c in range(NCORES):
        o = np.asarray(res.results[c]["out"]).astype(np.float32)
        toks, tok0, tok1, len0 = scatters[c]
        result[toks] = o[:, :TPC].T
        if len0:
            result[tok0] += o[:, TPC:TPC + len0].T
        if len(tok1):
            s1 = TPC + cap0 + (len0 - base1)
            result[tok1] += o[:, s1:s1 + len(tok1)].T
    return result.reshape(B, S, D)


# revision 32
# speedup vs baseline: 1.0151x; 1.0095x over previous
"""AdaptiveFractalFeedForward Trainium2 kernel (8 NeuronCores).

Strategy:
  - Token sharding: core c owns the tokens of one expert level plus a
    filler slice of a "split" level, exactly 512 tokens. The adapter
    therefore operates on the SAME per-core token set as the main MLP:
    slot0 (own level) covers columns [0, cap0), slot1 (split level)
    covers [base1, 512). Windows overlap; the host discards the
    columns that don't belong to each slot.
  - Main MLP bf16; adapter fp8(e4m3) with DoubleRow matmuls (2x PE)
    for slot0. Adapter output scales by mix ~5e-4 so fp8 error is
    negligible. Adapter weights pre-scaled by 8 (dodges fp8
    subnormals), un-scaled via mix/64.
  - LayerNorm without transposes: host sends x^T (feature-major);
    token mean/var come from PE ones-matmuls (sums of x and x^2 land
    pre-broadcast across partitions; eps via a K=1 accumulate).
    rstd = bf16 bit-trick rsqrt + 1 Newton step on the DVE; normalize
    is 2 batched DVE ops. Device computes -x_norm; host negates W1/A1.
  - PE warmup matmuls + warm-keeper trickle so the HAM clock gate
    stays at 2.4GHz through the stats chain.
  - bf16 partial outputs; host combines in fp32.
"""

import math
from contextlib import ExitStack

import ml_dtypes
import numpy as np

import concourse.bass as bass
import concourse.mybir as mybir
import concourse.tile as tile
from concourse import bacc
from concourse.bass_utils import run_bass_kernel_spmd
from concourse.tile_rust import add_dep_helper

B, S, D = 2, 2048, 768
HID, HID2 = 3072, 1536
NLEV = 9
NCORES = 8
TPC = (B * S) // NCORES  # 512 tokens per core
P = 128
KD = D // P        # 6
KH = HID // P      # 24
KH2 = HID2 // P    # 12
EPS = 1e-5
ASCALE = 8.0       # adapter weight pre-scale
NWARM = 11         # PE warmup matmuls
MAGIC16 = 0x5F37   # bf16 rsqrt magic

F32 = mybir.dt.float32
BF16 = mybir.dt.bfloat16
F8 = mybir.dt.float8e4
I16 = mybir.dt.int16
AF = mybir.ActivationFunctionType
AO = mybir.AluOpType
DR = mybir.MatmulPerfMode.DoubleRow

_PROGRAM_CACHE: dict = {}
LAST_EXEC_NS = None
LAST_RESULTS = None


def _build_program(cap0: int, base1: int, cap1: int):
    assert cap0 <= TPC and base1 + cap1 == TPC
    wout = TPC + cap0 + cap1

    nc = bacc.Bacc("TRN2", target_bir_lowering=False, debug=False,
                   num_devices=NCORES)

    xmT = nc.dram_tensor("xmT", [P, KD, TPC], BF16, kind="ExternalInput").ap()
    w1 = nc.dram_tensor("W1", [D, HID], BF16, kind="ExternalInput").ap()
    # W2 host-pretiled: [dt, p, kk, di] = W2[kk*128+p, dt*128+di]
    w2t = nc.dram_tensor("W2t", [KD, P, KH, P], BF16,
                         kind="ExternalInput").ap()
    # A1 host layout: [s, p, kk, h] = -8*A1_eff[s][kk*128+p, h]  (fp8)
    a1g = nc.dram_tensor("A1g", [2, P, KD, HID2], F8,
                         kind="ExternalInput").ap()
    # A2 host layout: [s, p, dt, kk, m] = 8*A2[s][kk*128+p, dt*128+m] (fp8)
    a2gt = nc.dram_tensor("A2gt", [2, P, KD, KH2, P], F8,
                          kind="ExternalInput").ap()
    b1v = nc.dram_tensor("b1", [P, KH], F32, kind="ExternalInput").ap()
    b2v = nc.dram_tensor("b2", [P, KD], F32, kind="ExternalInput").ap()
    a1bg = nc.dram_tensor("a1bg", [P, 2, KH2], F32, kind="ExternalInput").ap()
    a2bg = nc.dram_tensor("a2bg", [P, 2, KD], F32, kind="ExternalInput").ap()
    ommb = nc.dram_tensor("ommb", [P, TPC], F32, kind="ExternalInput").ap()
    mixab = nc.dram_tensor("mixab", [P, TPC], F32, kind="ExternalInput").ap()
    out = nc.dram_tensor("out", [D, wout], BF16, kind="ExternalOutput").ap()

    segs0 = [(0, cap0)]
    segs1 = [(base1, cap1)]

    with tile.TileContext(nc) as tc, ExitStack() as ctx, \
            nc.allow_low_precision(reason="bf16 LN stats are within budget"):
        singles = ctx.enter_context(tc.tile_pool(name="singles", bufs=1))
        wpool = ctx.enter_context(tc.tile_pool(name="wpool", bufs=3))
        w2pool = ctx.enter_context(tc.tile_pool(name="w2pool", bufs=6))
        opool = ctx.enter_context(tc.tile_pool(name="opool", bufs=6))
        sqpool = ctx.enter_context(tc.tile_pool(name="sqpool", bufs=1))
        vpool = ctx.enter_context(tc.tile_pool(name="vpool", bufs=2))
        pacc = ctx.enter_context(tc.tile_pool(name="pacc", bufs=3,
                                              space="PSUM"))
        pout = ctx.enter_context(tc.tile_pool(name="pout", bufs=3,
                                              space="PSUM"))
        psum_s = ctx.enter_context(tc.tile_pool(name="psum_s", bufs=2,
                                                space="PSUM"))

        def chain(op, prev, why="queue order"):
            if prev is not None:
                add_dep_helper(op.ins, prev.ins, reason=why)
            return op

        def bmid(ap, n):
            """Broadcast a [P, W] AP across a middle free dim of size n."""
            return bass.AP(tensor=ap.tensor, offset=ap.offset,
                           ap=[ap.ap[0], [0, n], ap.ap[1]])

        # ---- earliest DMA: xmT alone on the SP ring (2 unchained chunks
        # so squares can start on chunk 0) ----
        CHUNKS = [6, 6, 6, 6]
        w1_r = w1.rearrange("(t p) h -> p t h", p=P)

        xmT_sb = singles.tile([P, KD, TPC], BF16)
        nc.sync.dma_start(out=xmT_sb[:, 0:3, :], in_=xmT[:, 0:3, :])
        d_xmt = nc.sync.dma_start(out=xmT_sb[:, 3:6, :], in_=xmT[:, 3:6, :])
        d_sp = d_xmt

        # SWDGE: W1c0 held until xmT is through, then smalls
        w1c0 = wpool.tile([P, KD, 6 * P], BF16, tag="wa")
        d_gp = chain(nc.gpsimd.dma_start(out=w1c0, in_=w1_r[:, :, 0:6 * P]),
                     d_xmt, "W1c0 after xmT (startup bw)")
        b1_sb = singles.tile([P, KH], F32)
        d_gp = chain(nc.gpsimd.dma_start(out=b1_sb, in_=b1v), d_gp)
        b2_sb = singles.tile([P, KD], F32)
        d_gp = chain(nc.gpsimd.dma_start(out=b2_sb, in_=b2v), d_gp)
        a1b_sb = singles.tile([P, 2, KH2], F32)
        d_gp = chain(nc.gpsimd.dma_start(out=a1b_sb, in_=a1bg), d_gp)
        a2b_sb = singles.tile([P, 2, KD], F32)
        d_gp = chain(nc.gpsimd.dma_start(out=a2b_sb, in_=a2bg), d_gp)
        omm_sb = singles.tile([P, TPC], F32)
        d_gp = chain(nc.gpsimd.dma_start(out=omm_sb, in_=ommb), d_gp)
        mixa_sb = singles.tile([P, TPC], F32)
        d_gp = chain(nc.gpsimd.dma_start(out=mixa_sb, in_=mixab), d_gp)

        # ---- PE warmup ----
        # ones_t doubles as the 2^-10-scaled summing vector for LN stats
        ones_t = singles.tile([P, P], BF16)
        nc.vector.memset(ones_t, 2.0 ** -10)
        # eps row for the K=1 accumulate: eps*(3/4)*1024
        edm = singles.tile([1, TPC], BF16)
        nc.vector.memset(edm, EPS * 0.75 * 1024.0)
        warm_t = singles.tile([P, 256], BF16)
        nc.vector.memset(warm_t, 0.125)
        for i in range(NWARM):
            wp = pacc.tile([P, 256], F32, tag="acc")
            nc.tensor.matmul(wp, warm_t[:, 0:P], warm_t, start=True,
                             stop=True)

        # persistent activations
        xm_t = singles.tile([P, KD, TPC], BF16)   # -x_norm^T (bf16)
        xa8 = singles.tile([P, KD, TPC], F8)      # -x_norm^T (fp8)
        h_sb = singles.tile([P, KH, TPC], BF16)   # gelu(h)
        hl_sb = singles.tile([P, KH2, TPC], F8)   # relu(hl)*8 (fp8)

        C43 = 1024.0 / D  # sums are pre-scaled by 2^-10 via ones_t

        def trickle(after):
            wp2 = pacc.tile([P, 256], F32, tag="acc")
            wmm = nc.tensor.matmul(wp2, warm_t[:, 0:P], warm_t, start=True,
                                   stop=True)
            add_dep_helper(wmm.ins, after.ins, reason="HAM warm-keeper")

        # ---- LN stats + normalize: xm_t = (m - x) * rstd  (negated) ----
        rs_bc = singles.tile([P, TPC], BF16)
        tmp_n = sqpool.tile([P, KD, TPC], BF16, tag="nt")
        sq = sqpool.tile([P, KD, TPC], BF16, tag="sq")
        v0 = nc.vector.tensor_mul(out=sq[:, 0:3, :], in0=xmT_sb[:, 0:3, :],
                                  in1=xmT_sb[:, 0:3, :])
        for kq in range(3, 6):
            v0 = chain(nc.vector.tensor_mul(out=sq[:, kq:kq + 1, :],
                                            in0=xmT_sb[:, kq:kq + 1, :],
                                            in1=xmT_sb[:, kq:kq + 1, :]), v0)
        ps1 = psum_s.tile([P, TPC], F32, tag="s")
        ps2 = psum_s.tile([P, TPC], F32, tag="s")
        for kk in range(KD):
            nc.tensor.matmul(ps1, ones_t, xmT_sb[:, kk, :],
                             start=(kk == 0), stop=(kk == KD - 1))
            nc.tensor.matmul(ps2, ones_t, sq[:, kk, :],
                             start=(kk == 0), stop=False)
        xm_sum_last = nc.tensor.matmul(ps2, ones_t[0:1, :], edm[0:1, :],
                                       start=False, stop=True)
        # m, E[x^2] to bf16 SBUF on ACT (keeps the DVE chain in 2x mode)
        m_sb = vpool.tile([P, TPC], BF16, tag="vb")
        nc.scalar.activation(out=m_sb, in_=ps1, func=AF.Copy, scale=C43)
        v_t = vpool.tile([P, TPC], BF16, tag="v")
        nc.scalar.activation(out=v_t, in_=ps2, func=AF.Copy, scale=C43)
        msq = vpool.tile([P, TPC], BF16, tag="vb")
        last = chain(nc.vector.tensor_mul(out=msq, in0=m_sb, in1=m_sb), v0)
        last = chain(nc.vector.tensor_sub(out=v_t, in0=v_t, in1=msq), last)
        trickle(last)
        # rsqrt bit-trick on bf16 bits + 1 Newton step
        y = vpool.tile([P, TPC], BF16, tag="y")
        last = chain(nc.vector.tensor_scalar(
            out=y.bitcast(I16), in0=v_t.bitcast(I16), scalar1=1,
            scalar2=None, op0=AO.logical_shift_right), last)
        last = chain(nc.vector.tensor_scalar(
            out=y.bitcast(I16), in0=y.bitcast(I16), scalar1=-1,
            scalar2=MAGIC16, op0=AO.mult, op1=AO.add), last)
        t1 = vpool.tile([P, TPC], BF16, tag="vb")
        last = chain(nc.vector.tensor_mul(out=t1, in0=v_t, in1=y), last)
        last = chain(nc.vector.tensor_mul(out=t1, in0=t1, in1=y), last)
        last = chain(nc.vector.tensor_scalar(out=t1, in0=t1, scalar1=-0.5,
                                             scalar2=1.5, op0=AO.mult,
                                             op1=AO.add), last)
        last = chain(nc.vector.tensor_mul(out=rs_bc, in0=y, in1=t1), last)
        trickle(last)
        # normalize (negated): xm_t = (m - x) * rs
        last = chain(nc.vector.tensor_sub(out=tmp_n, in0=bmid(m_sb, KD),
                                          in1=xmT_sb), last)
        trickle(last)
        norm_last = chain(nc.vector.tensor_mul(out=xm_t, in0=tmp_n,
                                               in1=bmid(rs_bc, KD)), last)

        # fp8 copy for the adapter (off the critical path, during h)
        x8a = chain(nc.vector.tensor_copy(out=xa8[:, 0:3, :],
                                          in_=xm_t[:, 0:3, :]), norm_last)
        chain(nc.vector.tensor_copy(out=xa8[:, 3:6, :],
                                    in_=xm_t[:, 3:6, :]), x8a)

        # ---- remaining W1 on SP/ACT rings ----
        w1c1 = wpool.tile([P, KD, 6 * P], BF16, tag="wa")
        d_sp = chain(nc.sync.dma_start(out=w1c1,
                                       in_=w1_r[:, :, 6 * P:12 * P]), d_sp)

        # ---- phase A1: h = gelu(x_norm @ W1 + b1) ----
        W1ENG = [None, None, "scalar", "scalar"]
        d_act = None
        ht = 0
        gelu_first = None
        for ci, nch in enumerate(CHUNKS):
            if ci == 0:
                w1c = w1c0
            elif ci == 1:
                w1c = w1c1
            else:
                w1c = wpool.tile([P, KD, 6 * P], BF16, tag="wa")
                eng = getattr(nc, W1ENG[ci])
                c0 = sum(CHUNKS[:ci])
                dma = eng.dma_start(out=w1c[:, :, 0:nch * P],
                                    in_=w1_r[:, :, c0 * P:(c0 + nch) * P])
                if ci == 2:
                    add_dep_helper(dma.ins, d_xmt.ins,
                                   reason="W1c2 after xmT (startup bw)")
                else:
                    chain(dma, d_act)
                d_act = dma
            for j in range(nch):
                h_ps = pacc.tile([P, TPC], F32, tag="acc")
                for k in range(KD):
                    nc.tensor.matmul(h_ps, w1c[:, k, j * P:(j + 1) * P],
                                     xm_t[:, k, :], start=(k == 0),
                                     stop=(k == KD - 1))
                g = nc.scalar.activation(out=h_sb[:, ht, :], in_=h_ps,
                                         func=AF.Gelu,
                                         bias=b1_sb[:, ht:ht + 1])
                if gelu_first is None:
                    gelu_first = g
                ht += 1

        # ---- W2 loads (SP ring, all resident) ----
        w2cs = []
        for dt in range(KD):
            w2c = w2pool.tile([P, KH, P], BF16, tag="w2")
            d_sp = chain(nc.sync.dma_start(out=w2c, in_=w2t[dt]), d_sp)
            w2cs.append(w2c)

        # ---- A1 loads (SWDGE, held until the h phase is underway) ----
        a1_sb = singles.tile([P, 2, KD, HID2], F8)
        for s in range(2):
            d_gp = chain(nc.gpsimd.dma_start(out=a1_sb[:, s], in_=a1g[s]),
                         d_gp if s else gelu_first,
                         "A1 after h start (startup bw)")

        # ---- phase A2: main_out = (h @ W2 + b2) * (1-mix) ----
        for dt in range(KD):
            o_ps = pout.tile([P, TPC], F32, tag="po")
            for kk in range(KH):
                nc.tensor.matmul(o_ps, w2cs[dt][:, kk, :], h_sb[:, kk, :],
                                 start=(kk == 0), stop=(kk == KH - 1))
            o_sb = opool.tile([P, TPC], BF16, tag="osb")
            nc.vector.scalar_tensor_tensor(out=o_sb, in0=o_ps,
                                           scalar=b2_sb[:, dt:dt + 1],
                                           in1=omm_sb, op0=AO.add,
                                           op1=AO.mult)
            nc.scalar.dma_start(out=out[dt * P:(dt + 1) * P, 0:TPC], in_=o_sb)

        # ---- A2 prefetch (SP ring, after W2) ----
        a2_sb = singles.tile([P, 2, KD, KH2, P], F8)
        for s in range(2):
            d_sp = chain(nc.sync.dma_start(out=a2_sb[:, s], in_=a2gt[s]),
                         d_sp)

        # ---- phase B1: hl = relu(x_norm @ A1*8 + 8*a1b) ----
        for ht2 in range(KH2):
            for (sb, sl) in segs0:
                hl_ps = pacc.tile([P, TPC], F32, tag="acc")
                for k in range(KD // 2):
                    nc.tensor.matmul(
                        hl_ps[:, 0:sl],
                        a1_sb[:, 0, 2 * k:2 * k + 2, ht2 * P:(ht2 + 1) * P],
                        xa8[:, 2 * k:2 * k + 2, sb:sb + sl],
                        start=(k == 0), stop=(k == KD // 2 - 1),
                        perf_mode=DR)
                nc.scalar.activation(out=hl_sb[:, ht2, sb:sb + sl],
                                     in_=hl_ps[:, 0:sl], func=AF.Relu,
                                     bias=a1b_sb[:, 0, ht2:ht2 + 1])
            for (sb, sl) in segs1:
                hl_ps = pacc.tile([P, TPC], F32, tag="acc")
                for k in range(KD):
                    nc.tensor.matmul(hl_ps[:, 0:sl],
                                     a1_sb[:, 1, k, ht2 * P:(ht2 + 1) * P],
                                     xa8[:, k, sb:sb + sl],
                                     start=(k == 0), stop=(k == KD - 1))
                nc.scalar.activation(out=hl_sb[:, ht2, sb:sb + sl],
                                     in_=hl_ps[:, 0:sl], func=AF.Relu,
                                     bias=a1b_sb[:, 1, ht2:ht2 + 1])

        # ---- phase B2: adapter_out = (hl @ A2*8 + 64*a2b) * (mix/64) ----
        for dt in range(KD):
            for si, segs in enumerate((segs0, segs1)):
                col0 = TPC if si == 0 else TPC + cap0 - base1
                for (sb, sl) in segs:
                    if si == 0:
                        ao_ps = pout.tile([P, TPC], F32, tag="po")
                    else:
                        ao_ps = psum_s.tile([P, TPC], F32, tag="s")
                    if si == 0:
                        for k in range(KH2 // 2):
                            nc.tensor.matmul(
                                ao_ps[:, 0:sl],
                                a2_sb[:, 0, dt, 2 * k:2 * k + 2, :],
                                hl_sb[:, 2 * k:2 * k + 2, sb:sb + sl],
                                start=(k == 0), stop=(k == KH2 // 2 - 1),
                                perf_mode=DR)
                    else:
                        for kk in range(KH2):
                            nc.tensor.matmul(ao_ps[:, 0:sl],
                                             a2_sb[:, 1, dt, kk, :],
                                             hl_sb[:, kk, sb:sb + sl],
                                             start=(kk == 0),
                                             stop=(kk == KH2 - 1))
                    ao_sb = opool.tile([P, TPC], BF16, tag="osb")
                    nc.vector.scalar_tensor_tensor(
                        out=ao_sb[:, 0:sl], in0=ao_ps[:, 0:sl],
                        scalar=a2b_sb[:, si, dt:dt + 1],
                        in1=mixa_sb[:, sb:sb + sl], op0=AO.add, op1=AO.mult)
                    (nc.sync if si == 0 else nc.scalar).dma_start(
                        out=out[dt * P:(dt + 1) * P,
                                col0 + sb:col0 + sb + sl],
                        in_=ao_sb[:, 0:sl])

    nc.compile()
    return nc


def kernel(x, levels_info, gamma, beta, W1, b1, W2, b2, A1, a1b, A2, a2b,
           lmw, _trace=False, _trace_kwargs=None):
    global LAST_EXEC_NS, LAST_RESULTS
    x = np.ascontiguousarray(np.asarray(x, dtype=np.float32))
    levels_info = np.asarray(levels_info)
    gamma = np.asarray(gamma, dtype=np.float32)
    beta = np.asarray(beta, dtype=np.float32)
    W1 = np.asarray(W1, dtype=np.float32)
    b1 = np.asarray(b1, dtype=np.float32)
    W2 = np.asarray(W2, dtype=np.float32)
    b2 = np.asarray(b2, dtype=np.float32)
    A1 = np.asarray(A1, dtype=np.float32)
    a1b = np.asarray(a1b, dtype=np.float32)
    A2 = np.asarray(A2, dtype=np.float32)
    a2b = np.asarray(a2b, dtype=np.float32)
    lmw = np.asarray(lmw, dtype=np.float32)

    bf = ml_dtypes.bfloat16
    f8 = ml_dtypes.float8_e4m3

    xflat = x.reshape(B * S, D)

    # softmax over the sequence axis of lmw[depths] (shared across batch)
    depths = np.clip(levels_info[:, 0].astype(np.int64), 0, NLEV - 1)
    vals = lmw[depths]
    e = np.exp((vals - vals.max()).astype(np.float32))
    mix_pos = (e / e.sum()).astype(np.float32)  # [S]
    mix_flat = np.concatenate([mix_pos, mix_pos])  # [B*S]
    dflat = np.concatenate([depths, depths])

    # ---- pick the split level minimizing the slot-window overlap ----
    buckets = [np.nonzero(dflat == l)[0] for l in range(NLEV)]
    sizes = np.array([len(b) for b in buckets])
    best = None
    for l in range(NLEV):
        others = [i for i in range(NLEV) if i != l]
        mx = int(sizes[others].max())
        mn = int(sizes[others].min())
        if mx > TPC or mn == 0:
            continue
        cost = mx + TPC - mn  # cap0 + cap1
        if best is None or cost < best[0]:
            best = (cost, l, mx, mn)
    _, lsplit, cap0, base1 = best
    cap1 = TPC - base1
    others = [i for i in range(NLEV) if i != lsplit]

    key = (cap0, base1, cap1)
    if key not in _PROGRAM_CACHE:
        _PROGRAM_CACHE[key] = _build_program(cap0, base1, cap1)
    nc = _PROGRAM_CACHE[key]

    # ---- shared (replicated) host tensors ----
    # LayerNorm affine folded into first-layer weights; device computes
    # MINUS x_norm, so the x-side weights are negated.
    w1_eff = gamma[:, None] * W1
    b1_eff = (b1 + beta @ W1).astype(np.float32)
    w1_host = (-w1_eff).astype(bf)
    w2t_host = np.ascontiguousarray(
        W2.reshape(KH, P, KD, P).transpose(2, 1, 0, 3).astype(bf))
    A1_eff = -(gamma[None, :, None] * A1) * ASCALE
    a1b_eff = ((a1b + np.einsum("d,ldh->lh", beta, A1)) * ASCALE).astype(
        np.float32)
    A2_s = A2 * ASCALE
    a2b_s = (a2b * ASCALE * ASCALE).astype(np.float32)

    xflat_bf = xflat.astype(bf)
    b1_host = np.ascontiguousarray(b1_eff.reshape(KH, P).T)
    b2_host = np.ascontiguousarray(b2.reshape(KD, P).T)

    def xT_pack(tokens_2d):
        # [N, D] -> [P, KD, N] with [p, kk, t] = x[t, kk*128+p]
        n = tokens_2d.shape[0]
        return np.ascontiguousarray(
            tokens_2d.T.reshape(KD, P, n).transpose(1, 0, 2))

    # partition the split level into per-core fills
    fill_needed = [TPC - int(sizes[others[c]]) for c in range(NCORES)]
    assert sum(fill_needed) == int(sizes[lsplit])
    fill_tok = []
    off = 0
    for c in range(NCORES):
        fill_tok.append(buckets[lsplit][off:off + fill_needed[c]])
        off += fill_needed[c]

    in_maps = []
    scatters = []
    for c in range(NCORES):
        lvl0 = int(others[c])
        tok0 = buckets[lvl0]
        tok1 = fill_tok[c]
        toks = np.concatenate([tok0, tok1])
        len0 = len(tok0)

        a1g_c = np.ascontiguousarray(np.stack([
            A1_eff[lvl0].reshape(KD, P, HID2).transpose(1, 0, 2),
            A1_eff[lsplit].reshape(KD, P, HID2).transpose(1, 0, 2),
        ]).astype(f8))  # [2, P, KD, HID2]
        a2g_c = np.ascontiguousarray(np.stack([
            A2_s[lvl0].reshape(KH2, P, KD, P).transpose(1, 2, 0, 3),
            A2_s[lsplit].reshape(KH2, P, KD, P).transpose(1, 2, 0, 3),
        ]).astype(f8))  # [2, P, KD, KH2, P]

        in_maps.append({
            "xmT": xT_pack(xflat_bf[toks]),
            "W1": w1_host,
            "W2t": w2t_host,
            "A1g": a1g_c,
            "A2gt": a2g_c,
            "b1": b1_host,
            "b2": b2_host,
            "a1bg": np.ascontiguousarray(
                np.stack([a1b_eff[lvl0], a1b_eff[lsplit]])
                .reshape(2, KH2, P).transpose(2, 0, 1).astype(np.float32)),
            "a2bg": np.ascontiguousarray(
                np.stack([a2b_s[lvl0], a2b_s[lsplit]])
                .reshape(2, KD, P).transpose(2, 0, 1).astype(np.float32)),
            "ommb": np.ascontiguousarray(np.broadcast_to(
                (1.0 - mix_flat[toks]).astype(np.float32), (P, TPC))),
            "mixab": np.ascontiguousarray(np.broadcast_to(
                (mix_flat[toks] / (ASCALE * ASCALE)).astype(np.float32),
                (P, TPC))),
        })
        scatters.append((toks, tok0, tok1, len0))

    res = run_bass_kernel_spmd(nc, in_maps, core_ids=list(range(NCORES)),
                               trace=_trace, **(_trace_kwargs or {}))
    LAST_EXEC_NS = res.exec_time_ns
    LAST_RESULTS = res

    # ---- unshard: main part + additive adapter part ----
    result = np.zeros((B * S, D), dtype=np.float32)
    for c in range(NCORES):
        o = np.asarray(res.results[c]["out"]).astype(np.float32)
        toks, tok0, tok1, len0 = scatters[c]
        result[toks] = o[:, :TPC].T
        if len0:
            result[tok0] += o[:, TPC:TPC + len0].T
        if len(tok1):
            s1 = TPC + cap0 + (len0 - base1)
            result[tok1] += o[:, s1:s1 + len(tok1)].T
    return result.reshape(B, S, D)
